# revision 5
# baseline (speedup 1.0000x reference)
"""ChainClassifier TRN2 Bass kernel.

Data-parallel over the 8192 tokens across 8 NeuronCores (1024 tokens each),
weights replicated. Per core, the per-note base projection
base_n = x @ W1n[:, :512].T is chain-independent and runs as a dense bf16
hi/lo-split 3-pass matmul stream (near-fp32 accuracy at 3 cycles/row). The
sequential 88-note chain (window MLP -> correction -> relu -> w2 reduction
-> Bernoulli sample) runs in emb-major layout pipelined against the next
note's base matmuls. Sampling compares the logit against host-precomputed
thresholds sigma^-1(u) - b2 (no sigmoid on the critical path); probabilities
for prob_mass are computed off-path and reduced in one batched pass at the
end.

Hardware notes: compute-engine SBUF accesses must start at partition base
0/32/64/96, so per-note rows (ring slot, preds/probs row n) are written via
casting DMAs from partition-0 staging tiles, and hi/lo stacked operands are
padded to quad boundaries with zero weight rows in the matmul lhsT.
"""
import numpy as np
import ml_dtypes
from contextlib import ExitStack

import concourse.bass as bass
import concourse.mybir as mybir
import concourse.tile as tile
from concourse import bacc
from concourse.bass_utils import run_bass_kernel_spmd
from concourse.masks import make_identity

F32 = mybir.dt.float32
BF16 = mybir.dt.bfloat16
BF = ml_dtypes.bfloat16
Alu = mybir.AluOpType
Act = mybir.ActivationFunctionType

NUM_NOTES = 88
X_DIM = 512
EMB = 256
B, S = 8, 1024
N_CORES = 8
TPC = (B * S) // N_CORES          # tokens per core = 1024
HALF = TPC // 2                   # 512
KT = X_DIM // 128                 # 4 k-tiles

_NC_CACHE = {}


def _split(x):
    hi = np.asarray(x).astype(BF)
    lo = (np.asarray(x, np.float32) - hi.astype(np.float32)).astype(BF)
    return hi, lo


def _build_nc():
    nc = bacc.Bacc(None, target_bir_lowering=False, debug=False)

    xt_hi = nc.declare_dram_parameter("xt_hi", [X_DIM, TPC], BF16, isOutput=False)
    xt_lo = nc.declare_dram_parameter("xt_lo", [X_DIM, TPC], BF16, isOutput=False)
    w1h = nc.declare_dram_parameter("w1h", [NUM_NOTES, X_DIM, EMB], BF16, isOutput=False)
    w1l = nc.declare_dram_parameter("w1l", [NUM_NOTES, X_DIM, EMB], BF16, isOutput=False)
    w1lp = nc.declare_dram_parameter("w1lp", [NUM_NOTES, 44, 2 * EMB], BF16, isOutput=False)
    wc1rot = nc.declare_dram_parameter("wc1rot", [13, 288], BF16, isOutput=False)
    l2w = nc.declare_dram_parameter("l2w", [44, 24], BF16, isOutput=False)
    w2cols = nc.declare_dram_parameter("w2cols", [128, NUM_NOTES * 4], BF16, isOutput=False)
    b1cols = nc.declare_dram_parameter("b1cols", [128, NUM_NOTES * 2], F32, isOutput=False)
    b2row = nc.declare_dram_parameter("b2row", [1, NUM_NOTES], F32, isOutput=False)
    thresh = nc.declare_dram_parameter("thresh", [NUM_NOTES, TPC], F32, isOutput=False)
    onesrow = nc.declare_dram_parameter("onesrow", [1, HALF], BF16, isOutput=False)

    labels_o = nc.declare_dram_parameter("labels", [TPC, NUM_NOTES], F32, isOutput=True)
    pm_o = nc.declare_dram_parameter("pm", [1, TPC], F32, isOutput=True)

    with tile.TileContext(nc) as tc, ExitStack() as ctx:
        # ----- persistent SBUF -----
        st = ctx.enter_context(tc.tile_pool(name="static", bufs=1))

        xth_sb = st.tile([128, KT, TPC], BF16, tag="xth")
        xtl_sb = st.tile([128, KT, TPC], BF16, tag="xtl")
        for k in range(KT):
            nc.gpsimd.dma_start(out=xth_sb[:, k], in_=xt_hi[128 * k:128 * (k + 1), :])
            nc.gpsimd.dma_start(out=xtl_sb[:, k], in_=xt_lo[128 * k:128 * (k + 1), :])

        rot_sb = st.tile([13, 288], BF16, tag="rot")
        nc.gpsimd.dma_start(out=rot_sb, in_=wc1rot[:])
        l2w_sb = st.tile([44, 24], BF16, tag="l2w")
        nc.gpsimd.dma_start(out=l2w_sb, in_=l2w[:])
        w2_sb = st.tile([128, NUM_NOTES * 4], BF16, tag="w2")
        nc.gpsimd.dma_start(out=w2_sb, in_=w2cols[:])
        b1_sb = st.tile([128, NUM_NOTES * 2], F32, tag="b1")
        nc.gpsimd.dma_start(out=b1_sb, in_=b1cols[:])
        b2_sb = st.tile([1, NUM_NOTES], F32, tag="b2")
        nc.gpsimd.dma_start(out=b2_sb, in_=b2row[:])

        # ring: row 0 = ones, rows 1..12 = window slots (slot r at row 1+r)
        ring_sb = st.tile([13, TPC], BF16, tag="ring")
        nc.vector.memset(ring_sb[0:13], 0.0)
        nc.vector.memset(ring_sb[0:1], 1.0)

        # preds/probs, one bf16 row per note (written by casting DMAs)
        preds_sb = st.tile([NUM_NOTES, TPC], BF16, tag="preds")
        probs_sb = st.tile([NUM_NOTES, TPC], BF16, tag="probs")

        # r1: rows [relu1_hi(0:12) | ones(12) | zeros(13:32) | relu1_lo(32:44)]
        r1_sb = [st.tile([44, HALF], BF16, tag=f"r1_{h}", name=f"r1_{h}")
                 for h in range(2)]
        # lsp: rows [l_hi(0:12) | zeros(12:32) | l_lo(32:44)]
        lsp_sb = [st.tile([44, HALF], BF16, tag=f"lsp_{h}", name=f"lsp_{h}")
                  for h in range(2)]
        for h in range(2):
            nc.vector.memset(r1_sb[h][:], 0.0)
            nc.gpsimd.dma_start(out=r1_sb[h][12:13, :], in_=onesrow[:])
            nc.vector.memset(lsp_sb[h][:], 0.0)

        ident_sb = st.tile([128, 128], F32, tag="ident")
        make_identity(nc, ident_sb)
        ones_sb = st.tile([NUM_NOTES, 1], F32, tag="ones88")
        nc.vector.memset(ones_sb, 1.0)
        c88_sb = st.tile([1, 1], F32, tag="c88")
        nc.vector.memset(c88_sb, float(NUM_NOTES))

        # ----- main chain loop -----
        with ExitStack() as loop_ctx:
            wpool = loop_ctx.enter_context(tc.tile_pool(name="wpool", bufs=2))
            apool = loop_ctx.enter_context(tc.tile_pool(name="apool", bufs=2))
            spool = loop_ctx.enter_context(tc.tile_pool(name="spool", bufs=3))
            pre_pool = loop_ctx.enter_context(
                tc.tile_pool(name="pre", bufs=2, space="PSUM"))
            sm_pool = loop_ctx.enter_context(
                tc.tile_pool(name="smps", bufs=1, space="PSUM"))

            for n in range(NUM_NOTES):
                s = n % 12
                wh = wpool.tile([128, KT, EMB], BF16, tag="wh")
                wl = wpool.tile([128, KT, EMB], BF16, tag="wl")
                wp = wpool.tile([44, 2 * EMB], BF16, tag="wp")
                for k in range(KT):
                    nc.gpsimd.dma_start(out=wh[:, k], in_=w1h[n, 128 * k:128 * (k + 1), :])
                    nc.gpsimd.dma_start(out=wl[:, k], in_=w1l[n, 128 * k:128 * (k + 1), :])
                nc.gpsimd.dma_start(out=wp, in_=w1lp[n])

                for h in range(2):
                    tok = slice(HALF * h, HALF * (h + 1))
                    # threshold row for this note/half, staged at partition 0
                    thr_st = spool.tile([1, HALF], F32, tag="thr",
                                        name=f"thr_{n}_{h}")
                    nc.gpsimd.dma_start(out=thr_st, in_=thresh[n, tok])

                    pre = [pre_pool.tile([128, HALF], F32, tag=f"pre{e}",
                                         name=f"pre{e}_{n}_{h}")
                           for e in range(2)]
                    # base: 3-pass bf16 split, 4 k-tiles, 2 emb tiles
                    for e in range(2):
                        emb = slice(128 * e, 128 * (e + 1))
                        i = 0
                        for k in range(KT):
                            for (wt, xt) in ((wh, xth_sb), (wh, xtl_sb), (wl, xth_sb)):
                                nc.tensor.matmul(pre[e], wt[:, k, emb], xt[:, k, tok],
                                                 start=(i == 0), stop=False)
                                i += 1

                    # l1 = Wc1 @ window + bc1 (rotated lhsT, ring rhs rows 0:13)
                    l1_ps = sm_pool.tile([12, HALF], F32, tag="l1")
                    nc.tensor.matmul(l1_ps, rot_sb[:, 12 * s:12 * (s + 1)],
                                     ring_sb[:, tok], start=True, stop=False)
                    nc.tensor.matmul(l1_ps, rot_sb[:, 144 + 12 * s:144 + 12 * (s + 1)],
                                     ring_sb[:, tok], start=False, stop=True)
                    # relu1 split
                    nc.scalar.activation(r1_sb[h][0:12], l1_ps, Act.Relu)
                    nc.vector.scalar_tensor_tensor(
                        r1_sb[h][32:44], l1_ps, 0.0, r1_sb[h][0:12],
                        Alu.max, Alu.subtract)

                    # l2 = Wc2 @ relu1 + bc2
                    l2_ps = sm_pool.tile([12, HALF], F32, tag="l2")
                    nc.tensor.matmul(l2_ps, l2w_sb[0:44, 0:12], r1_sb[h][0:44],
                                     start=True, stop=False)
                    nc.tensor.matmul(l2_ps, l2w_sb[0:13, 12:24], r1_sb[h][0:13],
                                     start=False, stop=True)
                    # l split
                    nc.scalar.copy(lsp_sb[h][0:12], l2_ps)
                    nc.vector.scalar_tensor_tensor(
                        lsp_sb[h][32:44], l2_ps, 0.0, lsp_sb[h][0:12],
                        Alu.bypass, Alu.subtract)

                    # corr accumulated into pre
                    for e in range(2):
                        emb = slice(128 * e, 128 * (e + 1))
                        nc.tensor.matmul(pre[e], wp[0:44, emb], lsp_sb[h][0:44],
                                         start=False, stop=False)
                        nc.tensor.matmul(pre[e], wp[0:12, 256 + 128 * e:256 + 128 * (e + 1)],
                                         lsp_sb[h][0:12], start=False, stop=True)

                    # a = relu(pre + b1); split hi/lo; W2 reduction
                    logit_ps = sm_pool.tile([1, HALF], F32, tag="logit")
                    first = True
                    for e in range(2):
                        at = apool.tile([128, HALF], F32, tag=f"at{e}",
                                        name=f"at{e}_{n}_{h}")
                        ah = apool.tile([128, HALF], BF16, tag=f"ah{e}",
                                        name=f"ah{e}_{n}_{h}")
                        al = apool.tile([128, HALF], BF16, tag=f"al{e}",
                                        name=f"al{e}_{n}_{h}")
                        nc.scalar.activation(at, pre[e], Act.Relu,
                                             bias=b1_sb[:, 2 * n + e:2 * n + e + 1])
                        nc.vector.tensor_copy(ah, at)
                        nc.vector.tensor_tensor(al, at, ah, Alu.subtract)
                        chi = 4 * n + 2 * e
                        clo = chi + 1
                        for (c, a_t) in ((chi, ah), (chi, al), (clo, ah)):
                            nc.tensor.matmul(logit_ps, w2_sb[:, c:c + 1], a_t,
                                             start=first,
                                             stop=(e == 1 and c == clo))
                            first = False

                    # pred: logit > thresh -> staging, then DMA into place
                    p_st = spool.tile([1, HALF], F32, tag="pst",
                                      name=f"pst_{n}_{h}")
                    nc.vector.tensor_tensor(p_st, logit_ps, thr_st, Alu.is_gt)
                    nc.gpsimd.dma_start(out=ring_sb[1 + s:2 + s, tok], in_=p_st[:])
                    nc.gpsimd.dma_start(out=preds_sb[n:n + 1, tok], in_=p_st[:])
                    # prob (off critical path): sigmoid(logit + b2)
                    q_st = spool.tile([1, HALF], F32, tag="qst",
                                      name=f"qst_{n}_{h}")
                    nc.scalar.activation(q_st, logit_ps, Act.Sigmoid,
                                         bias=b2_sb[:, n:n + 1])
                    nc.gpsimd.dma_start(out=probs_sb[n:n + 1, tok], in_=q_st[:])

        # ----- epilogue: prob_mass + label transpose -----
        with ExitStack() as end_ctx:
            esb = end_ctx.enter_context(tc.tile_pool(name="esb", bufs=1))
            eps = end_ctx.enter_context(
                tc.tile_pool(name="eps", bufs=2, space="PSUM"))

            predsf = esb.tile([NUM_NOTES, TPC], F32, tag="predsf")
            nc.vector.tensor_copy(predsf, preds_sb)
            probsf = esb.tile([NUM_NOTES, TPC], F32, tag="probsf")
            nc.vector.tensor_copy(probsf, probs_sb)

            t1 = esb.tile([NUM_NOTES, TPC], F32, tag="t1")
            nc.vector.tensor_tensor(t1, predsf, probsf, Alu.mult)
            v = esb.tile([NUM_NOTES, TPC], F32, tag="v")
            nc.vector.scalar_tensor_tensor(v, t1, 2.0, probsf,
                                           Alu.mult, Alu.subtract)
            u = esb.tile([NUM_NOTES, TPC], F32, tag="u")
            nc.vector.tensor_tensor(u, v, predsf, Alu.subtract)

            pm_ps = eps.tile([1, TPC], F32, tag="pm")
            nc.tensor.matmul(pm_ps[:, 0:HALF], ones_sb, u[:, 0:HALF],
                             start=True, stop=True)
            nc.tensor.matmul(pm_ps[:, HALF:TPC], ones_sb, u[:, HALF:TPC],
                             start=True, stop=True)
            pm_sb = esb.tile([1, TPC], F32, tag="pmsb")
            nc.scalar.activation(pm_sb, pm_ps, Act.Identity, bias=c88_sb[:])
            nc.gpsimd.dma_start(out=pm_o[:], in_=pm_sb)

            for b in range(TPC // 128):
                tp_ps = eps.tile([128, NUM_NOTES], F32, tag="tp",
                                 name=f"tp_{b}")
                nc.tensor.transpose(tp_ps, predsf[:, 128 * b:128 * (b + 1)],
                                    ident_sb[0:NUM_NOTES, 0:NUM_NOTES])
                ob = esb.tile([128, NUM_NOTES], F32, tag="ob", name=f"ob_{b}")
                nc.vector.tensor_copy(ob, tp_ps)
                nc.gpsimd.dma_start(out=labels_o[128 * b:128 * (b + 1), :], in_=ob)

    nc.compile()
    return nc


def _host_prep(x, Wc1, bc1, Wc2, bc2, W1, b1, W2, b2, noise):
    """Build per-core input maps (all numpy)."""
    x = np.asarray(x, np.float32)
    W1 = np.asarray(W1, np.float32)
    noise = np.asarray(noise, np.float32)
    Wc1 = np.asarray(Wc1, np.float32)
    Wc2 = np.asarray(Wc2, np.float32)
    bc1 = np.asarray(bc1, np.float32)
    bc2 = np.asarray(bc2, np.float32)
    b1 = np.asarray(b1, np.float32)
    W2 = np.asarray(W2, np.float32)
    b2 = np.asarray(b2, np.float32)

    # W1 x-part, transposed to [n, d, e], split hi/lo
    w1xt = np.ascontiguousarray(W1[:, :, :X_DIM].transpose(0, 2, 1))
    w1h, w1l = _split(w1xt)

    # W1 l-part lhsT blocks [n, 44, 512]:
    #   cols 0-255 (A): rows 0-11 = W1lT_hi, rows 32-43 = W1lT_hi (pairs l_lo)
    #   cols 256-511 (B): rows 0-11 = W1lT_lo (pairs l_hi)
    w1lT = np.ascontiguousarray(W1[:, :, X_DIM:].transpose(0, 2, 1))  # [n, 12, 256]
    w1lT_h, w1lT_l = _split(w1lT)
    w1lp = np.zeros((NUM_NOTES, 44, 2 * EMB), BF)
    w1lp[:, 0:12, 0:EMB] = w1lT_h
    w1lp[:, 32:44, 0:EMB] = w1lT_h
    w1lp[:, 0:12, EMB:] = w1lT_l

    # Wc1 rotations + bc1 at row 0 (pairs ring ones row):
    # lhsT[0, i] = bc1[i]; lhsT[1+r, i] = Wc1[i, (r - s) % 12]
    wc1h, wc1l = _split(Wc1)
    bc1h, bc1l = _split(bc1)
    wc1rot = np.zeros((13, 288), BF)
    for s in range(12):
        wc1rot[0, s * 12:(s + 1) * 12] = bc1h
        wc1rot[0, 144 + s * 12:144 + (s + 1) * 12] = bc1l
        for r in range(12):
            j = (r - s) % 12
            wc1rot[1 + r, s * 12:(s + 1) * 12] = wc1h[:, j]
            wc1rot[1 + r, 144 + s * 12:144 + (s + 1) * 12] = wc1l[:, j]

    # l2w [44, 24]:
    #   cols 0-11 (A, K=44): rows 0-11 = Wc2T_hi, row 12 = bc2_hi,
    #                        rows 32-43 = Wc2T_hi (pairs relu1_lo)
    #   cols 12-23 (B, K=13): rows 0-11 = Wc2T_lo, row 12 = bc2_lo
    wc2h, wc2l = _split(Wc2)
    bc2h, bc2l = _split(bc2)
    l2w = np.zeros((44, 24), BF)
    l2w[0:12, 0:12] = wc2h.T
    l2w[12, 0:12] = bc2h
    l2w[32:44, 0:12] = wc2h.T
    l2w[0:12, 12:24] = wc2l.T
    l2w[12, 12:24] = bc2l

    # w2 columns: [128, n*4 + e*2 + hl]
    w2h, w2l = _split(W2)
    w2cols = np.zeros((128, NUM_NOTES * 4), BF)
    for e in range(2):
        blk = slice(128 * e, 128 * (e + 1))
        w2cols[:, (4 * np.arange(NUM_NOTES) + 2 * e)] = w2h[:, blk].T
        w2cols[:, (4 * np.arange(NUM_NOTES) + 2 * e + 1)] = w2l[:, blk].T

    # b1 columns [128, n*2+e] fp32
    b1cols = np.zeros((128, NUM_NOTES * 2), np.float32)
    b1cols[:, 0::2] = b1[:, :128].T
    b1cols[:, 1::2] = b1[:, 128:].T

    b2row = b2.reshape(1, NUM_NOTES)

    # thresholds: sigma^-1(u) - b2, computed in fp64
    u = noise.reshape(NUM_NOTES, B * S).astype(np.float64)
    with np.errstate(divide="ignore"):
        t = np.log(u) - np.log1p(-u)
    t = np.clip(t, -1e30, 1e30) - b2.astype(np.float64)[:, None]
    thr = t.astype(np.float32)

    onesrow = np.ones((1, HALF), BF)

    xf = x.reshape(B * S, X_DIM)
    in_maps = []
    for c in range(N_CORES):
        sl = slice(TPC * c, TPC * (c + 1))
        xt = np.ascontiguousarray(xf[sl].T)
        xh, xl = _split(xt)
        in_maps.append({
            "xt_hi": xh, "xt_lo": xl,
            "w1h": w1h, "w1l": w1l, "w1lp": w1lp,
            "wc1rot": wc1rot, "l2w": l2w, "w2cols": w2cols,
            "b1cols": b1cols, "b2row": b2row,
            "thresh": np.ascontiguousarray(thr[:, sl]),
            "onesrow": onesrow,
        })
    return in_maps


def kernel(x, Wc1, bc1, Wc2, bc2, W1, b1, W2, b2, noise):
    in_maps = _host_prep(x, Wc1, bc1, Wc2, bc2, W1, b1, W2, b2, noise)
    if "nc" not in _NC_CACHE:
        _NC_CACHE["nc"] = _build_nc()
    nc = _NC_CACHE["nc"]
    res = run_bass_kernel_spmd(nc, in_maps, list(range(N_CORES)))
    labels = np.stack([res.results[c]["labels"] for c in range(N_CORES)])
    pm = np.stack([res.results[c]["pm"].reshape(TPC) for c in range(N_CORES)])
    return labels.astype(np.float32), pm.astype(np.float32)
